# revision 42
# baseline (speedup 1.0000x reference)
"""Trainium2 Bass kernel for nn_CSSMSHViT_60043642798201.

Strategy
--------
The reference repeats the input image over a time axis T=8 and runs a gated
scalar recurrence over T.  Because the input is constant over T the whole
temporal structure collapses algebraically:

    h_t = (1 - a^{t+1}) z          (closed form of the scan)

so the per-timestep fields are never materialised.  The kernel computes

  LN1 (global per-batch) -> +3x3 depthwise pos conv -> z/sigma/g projections
  -> power ladder u_t = a^t z with fused per-batch reductions St = sum(u_t*Gt)
     where Gt = DW5^T(g)  (adjoint trick: mean(DW5(u)*g) = mean(u*DW5^T(g)))
  -> tiny gate MLP -> softmax weights w
  -> F = z - sum_t w_t u_{t+1};  x_out = (DW5(F)+b_sp)*g @ W_out + b_out
  -> out1 = x + x_out -> LN2 -> MLP with 3x3 depthwise conv -> out.

Sharding: pure data-parallel over batch (32 = 8 cores x 4), no collectives.

Perf structure (v2):
  * the Q-ladder runs as fused tensor_tensor_reduce ops (multiply + per-batch
    segment sum in one DVE instr, reading conv PSUM directly) interleaved
    chunk-by-chunk with the 5x5 conv so the scan hides under PE conv time;
  * the Horner recurrence for W = sum_t w_t a^{t+1} folds the rho powers into
    the softmax weights so each step is one per-batch stt, and the final
    F = z*(1-W) is a single fused stt per batch;
  * MLP depthwise-conv diagonal weights are precomputed on host and DMAed;
  * LN2 stats are transposed into [128,8] via tiny data-stationary matmuls
    (avoids a 6.5us single-partition reciprocal);
  * LN1 squares via fused ttr; gate-MLP gelu via sigmoid approx (saves
    activation table loads).
"""

import numpy as np
import ml_dtypes

BF16 = ml_dtypes.bfloat16
FP8 = ml_dtypes.float8_e4m3

# problem constants
B, T, H, W, C = 32, 8, 16, 16, 384
KS = 5
HID = 4 * C
GH = max(C // 4, 8)
RHO = 0.999
EPS = 1e-6

NCORES = 8
BL = B // NCORES            # batches per core = 4
HWN = H * W                 # 256 tokens per image
NTOK = BL * HWN             # 1024 tokens per core
NCC = C // 128              # 3 channel chunks
NHC = HID // 128            # 12 hidden chunks

DW8S = 32.0                 # fp8 scale folded into the MLP dwconv kernel

# padded geometries (channel-major fields, free layout (b, hp, wp))
H1, W1P = 18, 18            # pad-1 buffers (3x3 convs)
F1 = BL * H1 * W1P
H2, W2P = 20, 20            # pad-2 buffers (5x5 convs)
F2 = BL * H2 * W2P

_PROG = None  # cached compiled program


def _build_program():
    import concourse.bass as bass
    import concourse.tile as tile
    from concourse import bacc, mybir

    fp32 = mybir.dt.float32
    bf16 = mybir.dt.bfloat16
    AF = mybir.ActivationFunctionType
    OP = mybir.AluOpType
    AX = mybir.AxisListType

    nc = bacc.Bacc("TRN2", target_bir_lowering=False)

    # ---------------- DRAM tensors ----------------
    d = {}
    d["x_hi"] = nc.dram_tensor("x_hi", [NTOK, C], bf16, kind="ExternalInput")
    d["x_lo"] = nc.dram_tensor("x_lo", [NTOK, C], bf16, kind="ExternalInput")
    # matmul weights, chunked [kchunks, 128, M] bf16
    d["w_in"] = nc.dram_tensor("w_in", [128, NCC, C], bf16, kind="ExternalInput")
    d["w_a"] = nc.dram_tensor("w_a", [128, NCC, C], bf16, kind="ExternalInput")
    d["w_g"] = nc.dram_tensor("w_g", [128, NCC, C], bf16, kind="ExternalInput")
    d["w_out"] = nc.dram_tensor("w_out", [128, NCC, C], bf16, kind="ExternalInput")
    d["w1"] = nc.dram_tensor("w1", [128, NCC, HID], bf16, kind="ExternalInput")
    d["w2"] = nc.dram_tensor("w2", [128, NHC, C], bf16, kind="ExternalInput")
    d["wg1"] = nc.dram_tensor("wg1", [128, 2 * NCC, GH], bf16, kind="ExternalInput")
    d["wg2"] = nc.dram_tensor("wg2", [GH, 1], bf16, kind="ExternalInput")
    # diagonalised depthwise kernels (partition-major, contiguous per partition)
    fp8 = mybir.dt.float8e4
    d["dpos"] = nc.dram_tensor("dpos", [128, 9, NCC, 128], bf16, kind="ExternalInput")
    d["dsp"] = nc.dram_tensor("dsp", [128, 25, NCC, 128], bf16, kind="ExternalInput")
    # MLP depthwise 3x3 kernel as fp8 diagonal tap-pairs (DoubleRow) + single
    d["ddw8"] = nc.dram_tensor("ddw8", [128, 4, NHC, 2, 128], fp8,
                               kind="ExternalInput")
    d["ddw1"] = nc.dram_tensor("ddw1", [128, NHC, 128], fp8, kind="ExternalInput")
    # per-channel vectors [128, nchunks] fp32
    for nm in ["b_in", "b_a", "b_g", "b_sp", "b_out", "b2", "gamma1", "beta1",
               "b_pos"]:
        d[nm] = nc.dram_tensor(nm, [128, NCC], fp32, kind="ExternalInput")
    d["b1"] = nc.dram_tensor("b1", [128, NHC], fp32, kind="ExternalInput")
    d["bdw"] = nc.dram_tensor("bdw", [128, NHC], fp32, kind="ExternalInput")
    d["g2r"] = nc.dram_tensor("g2r", [1, NCC, 128], bf16, kind="ExternalInput")
    d["be2"] = nc.dram_tensor("be2", [128, NCC], fp32, kind="ExternalInput")
    d["bg1"] = nc.dram_tensor("bg1", [GH, 1], fp32, kind="ExternalInput")
    d["bg2"] = nc.dram_tensor("bg2", [1, 1], fp32, kind="ExternalInput")
    d["prior"] = nc.dram_tensor("prior", [1, BL * T], fp32, kind="ExternalInput")
    d["rhopow"] = nc.dram_tensor("rhopow", [1, BL * T], fp32, kind="ExternalInput")
    out_d = nc.dram_tensor("out", [NTOK, C], fp32, kind="ExternalOutput")

    with tile.TileContext(nc) as tc:
        _emit(nc, tc, d, out_d, mybir, bass, fp32, bf16, fp8, AF, OP, AX)

    nc.compile()
    return nc


def _emit(nc, tc, d, out_d, mybir, bass, fp32, bf16, fp8, AF, OP, AX):
    import os
    SMAX = int(os.environ.get("BASS_SMAX", "99"))
    from contextlib import ExitStack
    ctx = ExitStack()

    pool = ctx.enter_context(tc.tile_pool(name="persist", bufs=1))
    scr = ctx.enter_context(tc.tile_pool(name="scratch", bufs=2))
    pp_mm = ctx.enter_context(tc.tile_pool(name="pp_mm", bufs=5, space="PSUM"))
    pp_tr = ctx.enter_context(tc.tile_pool(name="pp_tr", bufs=2, space="PSUM"))
    pp_sm = ctx.enter_context(tc.tile_pool(name="pp_sm", bufs=1, space="PSUM"))

    # ---------------- persistent field tiles ----------------
    x_cm = pool.tile([128, NCC, NTOK], fp32, name="x_cm")          # also final out
    xn0p = pool.tile([128, NCC, F1], bf16, name="xn0p")            # padded LN1 out
    xpos = pool.tile([128, NCC, NTOK], bf16, name="xpos")
    z_f = pool.tile([128, NCC, NTOK], bf16, name="z_f")            # reused as yn
    sg_f = pool.tile([128, NCC, NTOK], bf16, name="sg_f")
    g_p = pool.tile([128, NCC, F2], bf16, name="g_p")              # padded silu gate
    u_f = pool.tile([128, NCC, NTOK], bf16, name="u_f")            # ladder / acc / sq
    f_p = pool.tile([128, NCC, F2], bf16, name="f_p")              # padded F field
    out1 = pool.tile([128, NCC, NTOK], fp32, name="out1")          # also out staging
    h1p = pool.tile([128, NHC, F1], fp8, name="h1p")               # padded MLP hidden

    # weights
    w_in_t = pool.tile([128, NCC, C], bf16, name="w_in_t")
    w_a_t = pool.tile([128, NCC, C], bf16, name="w_a_t")
    w_g_t = pool.tile([128, NCC, C], bf16, name="w_g_t")
    w_out_t = pool.tile([128, NCC, C], bf16, name="w_out_t")
    w1_t = pool.tile([128, NCC, HID], bf16, name="w1_t")
    w2_t = pool.tile([128, NHC, C], bf16, name="w2_t")
    wg1_t = pool.tile([128, 2 * NCC, GH], bf16, name="wg1_t")
    wg2_t = pool.tile([GH, 1], bf16, name="wg2_t")
    dsp_t = pool.tile([128, 25, NCC, 128], bf16, name="dsp_t")     # resident 5x5 diags
    dpos_t = pool.tile([128, 9, NCC, 128], bf16, name="dpos_t")    # resident 3x3 diags
    ddw8_t = pool.tile([128, 4, NHC, 2, 128], fp8, name="ddw8_t")  # dw diag pairs
    ddw1_t = pool.tile([128, NHC, 128], fp8, name="ddw1_t")        # dw diag single

    # vectors
    b_in_c = pool.tile([128, NCC], fp32, name="b_in_c")
    b_a_c = pool.tile([128, NCC], fp32, name="b_a_c")
    b_g_c = pool.tile([128, NCC], fp32, name="b_g_c")
    b_sp_c = pool.tile([128, NCC], fp32, name="b_sp_c")
    b_pos_c = pool.tile([128, NCC], fp32, name="b_pos_c")
    b_out_c = pool.tile([128, NCC], fp32, name="b_out_c")
    b2_c = pool.tile([128, NCC], fp32, name="b2_c")
    g1_c = pool.tile([128, NCC], fp32, name="g1_c")
    be1_c = pool.tile([128, NCC], fp32, name="be1_c")
    b1_c = pool.tile([128, NHC], fp32, name="b1_c")
    bdw_c = pool.tile([128, NHC], fp32, name="bdw_c")
    g2_t = pool.tile([1, NCC, 128], bf16, name="g2_t")
    be2_c = pool.tile([128, NCC], fp32, name="be2_c")
    bg1_c = pool.tile([GH, 1], fp32, name="bg1_c")
    bg2_c = pool.tile([1, 1], fp32, name="bg2_c")
    prior_r = pool.tile([1, BL * T], fp32, name="prior_r")
    rhopow_r = pool.tile([1, BL * T], fp32, name="rhopow_r")

    # small working tiles
    ident = pool.tile([128, 128], bf16, name="ident")
    ones_c = pool.tile([128, 1], bf16, name="ones_c")    # holds 1/C (LN2 stats)
    sums = pool.tile([128, 24], fp32, name="sums")       # stat*12 + b*3 + kc
    ar = pool.tile([128, 24], fp32, name="ar")
    tot = pool.tile([128, 2, BL], fp32, name="tot")
    m_col = pool.tile([128, BL], fp32, name="m_col")
    e2_col = pool.tile([128, BL], fp32, name="e2_col")
    var_col = pool.tile([128, BL], fp32, name="var_col")
    rstd_col = pool.tile([128, BL], fp32, name="rstd_col")
    sc_col = pool.tile([128, NCC, BL], fp32, name="sc_col")
    bi_col = pool.tile([128, NCC, BL], fp32, name="bi_col")
    tmp_col = pool.tile([128, BL], fp32, name="tmp_col")
    st_all = pool.tile([128, NCC, BL, T], fp32, name="st_all")
    s0_c = pool.tile([128, NCC, BL], fp32, name="s0_c")
    gbar_c = pool.tile([128, NCC, BL], fp32, name="gbar_c")
    s0gb = pool.tile([128, NCC, BL], fp32, name="s0gb")
    kv = pool.tile([128, NCC, BL, T], bf16, name="kv")
    qt = pool.tile([128, NCC, BL, T], bf16, name="qt")
    kw = pool.tile([128, NCC, BL * T], bf16, name="kw")
    hg = pool.tile([GH, BL * T], bf16, name="hg")
    hgs = pool.tile([GH, BL * T], bf16, name="hgs")
    logits = pool.tile([1, BL * T], fp32, name="logits")
    esh = pool.tile([1, BL * T], fp32, name="esh")
    se_r = pool.tile([1, BL], fp32, name="se_r")
    wneg = pool.tile([1, BL * T], fp32, name="wneg")
    wbc = pool.tile([128, BL * T], fp32, name="wbc")
    # LN2 small tiles
    ln2b = pool.tile([128, 16], bf16, name="ln2b")       # rstd8 | -mu*rstd8 (bf16)
    ln2m = pool.tile([128, 8], fp32, name="ln2m")        # mu
    ln2v = pool.tile([128, 8], fp32, name="ln2v")        # var -> rstd
    rsb = pool.tile([16, 128], bf16, name="rsb")

    # ---------------- loads (x first; then in consumption order) ----------------
    stg = pool.tile([128, NTOK // 128, 2 * C], bf16, name="stg")
    xhi_s = stg[:, :, 0:C]
    xlo_s = stg[:, :, C:2 * C]
    nc.sync.dma_start(
        xhi_s, d["x_hi"][:].rearrange("(i p) c -> p i c", p=128))
    nc.sync.dma_start(
        xlo_s, d["x_lo"][:].rearrange("(i p) c -> p i c", p=128))

    def ld(tile_ap, dram):
        nc.sync.dma_start(tile_ap[:], dram[:])

    for nm, t_ in [("gamma1", g1_c), ("beta1", be1_c), ("b_pos", b_pos_c),
                   ("b_in", b_in_c), ("b_a", b_a_c), ("b_g", b_g_c),
                   ("b_sp", b_sp_c), ("b_out", b_out_c), ("b2", b2_c)]:
        ld(t_, d[nm])
    ld(dpos_t, d["dpos"])
    ld(w_g_t, d["w_g"])
    ld(w_in_t, d["w_in"])
    ld(w_a_t, d["w_a"])
    ld(dsp_t, d["dsp"])
    ld(w_out_t, d["w_out"])
    ld(wg1_t, d["wg1"])
    nc.sync.dma_start(wg2_t[:], d["wg2"][:])
    nc.sync.dma_start(g2_t[:], d["g2r"][:])
    ld(be2_c, d["be2"])
    nc.sync.dma_start(bg1_c[:], d["bg1"][:])
    nc.sync.dma_start(bg2_c[:], d["bg2"][:])
    nc.sync.dma_start(prior_r[:], d["prior"][:])
    nc.sync.dma_start(rhopow_r[:], d["rhopow"][:])
    ld(w1_t, d["w1"])
    ld(ddw8_t, d["ddw8"])
    ld(ddw1_t, d["ddw1"])
    ld(w2_t, d["w2"])
    ld(b1_c, d["b1"])
    ld(bdw_c, d["bdw"])

    from concourse.masks import make_identity
    make_identity(nc, ident[:])
    nc.vector.memset(ones_c[:], 1.0 / float(C))

    # zero padded buffers (borders must stay zero); xn0p first (needed soonest)
    nc.gpsimd.memset(xn0p[:].rearrange("p a b -> p (a b)"), 0.0)
    nc.gpsimd.memset(g_p[:].rearrange("p a b -> p (a b)"), 0.0)
    nc.gpsimd.memset(f_p[:].rearrange("p a b -> p (a b)"), 0.0)
    nc.gpsimd.memset(h1p[:].rearrange("p a b -> p (a b)"), 0.0)

    # view helpers -------------------------------------------------
    def pad1(tile_, j):           # -> [128, BL, H1, W1P] for chunk j
        return tile_[:, j, :].rearrange("p (b h w) -> p b h w", b=BL, h=H1, w=W1P)

    def pad2(tile_, j):
        return tile_[:, j, :].rearrange("p (b h w) -> p b h w", b=BL, h=H2, w=W2P)

    def dense(tile_, j):          # -> [128, BL, H, W]
        return tile_[:, j, :].rearrange("p (b h w) -> p b h w", b=BL, h=H, w=W)

    def int1(tile_, j):           # pad1 interior
        return pad1(tile_, j)[:, :, 1:1 + H, 1:1 + W]

    def int2(tile_, j):
        return pad2(tile_, j)[:, :, 2:2 + H, 2:2 + W]

    HV = NTOK // 512              # 2 halves (2 batches each)

    # ---------------- A+B: transpose x on PE, LN1 stats interleaved ----------
    sview = sums[:].rearrange("p (s b k) -> p s b k", s=2, b=BL, k=NCC)
    for kc in range(NCC):
        for i in range(NTOK // 128):
            pt = pp_tr.tile([128, 128], fp32, tag="tr", name=f"trx{i}_{kc}")
            nc.tensor.matmul(pt[:], xhi_s[:, i, kc * 128:(kc + 1) * 128],
                             ident[:], start=True, stop=False)
            nc.tensor.matmul(pt[:], xlo_s[:, i, kc * 128:(kc + 1) * 128],
                             ident[:], start=False, stop=True)
            nc.scalar.copy(x_cm[:, kc, i * 128:(i + 1) * 128], pt[:])
        if SMAX >= 2:
            nc.vector.tensor_reduce(
                sview[:, 0, :, kc],
                x_cm[:, kc, :].rearrange("p (b n) -> p b n", b=BL),
                axis=AX.X, op=OP.add)
            for b in range(BL):
                s_sc = scr.tile([128, HWN], bf16, tag="sq_scr", name=f"sxx{kc}{b}")
                nc.vector.scalar_tensor_tensor(
                    s_sc[:], x_cm[:, kc, b * HWN:(b + 1) * HWN], 0.0,
                    x_cm[:, kc, b * HWN:(b + 1) * HWN],
                    op0=OP.bypass, op1=OP.mult,
                    accum_out=sview[:, 1, b, kc:kc + 1])

    # ---------------- B: LN1 stats finalize + apply ----------------
    if SMAX >= 2:
        import concourse.bass_isa as bass_isa
        RADD = bass_isa.ReduceOp.add
        nc.gpsimd.partition_all_reduce(ar[:], sums[:], channels=128, reduce_op=RADD)
        nc.vector.tensor_reduce(
            tot[:], ar[:].rearrange("p (s b k) -> p s b k", s=2, b=BL, k=NCC),
            axis=AX.X, op=OP.add)
        NB = float(HWN * C)
        nc.vector.tensor_scalar(m_col[:], tot[:, 0, :], 1.0 / NB, None, op0=OP.mult)
        nc.vector.tensor_scalar(e2_col[:], tot[:, 1, :], 1.0 / NB, None, op0=OP.mult)
        nc.vector.tensor_tensor(tmp_col[:], m_col[:], m_col[:], op=OP.mult)
        nc.vector.tensor_tensor(var_col[:], e2_col[:], tmp_col[:], op=OP.subtract)
        nc.vector.tensor_scalar(var_col[:], var_col[:], EPS, None, op0=OP.add)
        nc.scalar.sqrt(var_col[:], var_col[:])
        nc.vector.reciprocal(rstd_col[:], var_col[:])
        for kc in range(NCC):
            nc.vector.tensor_scalar(
                sc_col[:, kc, :], rstd_col[:], g1_c[:, kc:kc + 1], None, op0=OP.mult)
            nc.vector.tensor_tensor(tmp_col[:], m_col[:], sc_col[:, kc, :], op=OP.mult)
            nc.vector.tensor_scalar(
                bi_col[:, kc, :], tmp_col[:], be1_c[:, kc:kc + 1], -1.0,
                op0=OP.subtract, op1=OP.mult)
            for b in range(BL):
                nc.scalar.activation(
                    pad1(xn0p, kc)[:, b, 1:1 + H, 1:1 + W],
                    dense(x_cm, kc)[:, b],
                    AF.Identity,
                    bias=bi_col[:, kc, b:b + 1], scale=sc_col[:, kc, b:b + 1])

    # ---------------- C: positional 3x3 conv -> xpos ----------------
    if SMAX >= 3:
        for kc in range(NCC):
            for hv in range(HV):
                ps = pp_mm.tile([128, 512], fp32, tag="mm", name=f"cpos{kc}{hv}")
                for ti, (i, j) in enumerate([(a, b) for a in range(3) for b in range(3)]):
                    rhs = pad1(xn0p, kc)[:, 2 * hv:2 * hv + 2, i:i + H, j:j + W]
                    nc.tensor.matmul(
                        ps[:], dpos_t[:, ti, kc, :], rhs,
                        start=(ti == 0), stop=(ti == 8))
                ps4 = ps[:].rearrange("p (b h w) -> p b h w", b=2, h=H, w=W)
                for bb in range(2):
                    b = 2 * hv + bb
                    nc.vector.scalar_tensor_tensor(
                        dense(xpos, kc)[:, b], ps4[:, bb],
                        b_pos_c[:, kc:kc + 1],
                        int1(xn0p, kc)[:, b],
                        op0=OP.add, op1=OP.add)

    # ---------------- D: z / sigma / g projections ----------------
    if SMAX >= 4:
        def mm_c(dst_evac, w_t, hv_count=HV):
            for mc in range(NCC):
                for hv in range(hv_count):
                    ps = pp_mm.tile([128, 512], fp32, tag="mm",
                                    name=f"mmc_{id(w_t)}_{mc}_{hv}")
                    for kc in range(NCC):
                        nc.tensor.matmul(
                            ps[:], w_t[:, kc, mc * 128:(mc + 1) * 128],
                            xpos[:, kc, hv * 512:(hv + 1) * 512],
                            start=(kc == 0), stop=(kc == NCC - 1))
                    dst_evac(mc, hv, ps)

        def evac_z(mc, hv, ps):
            nc.scalar.activation(z_f[:, mc, hv * 512:(hv + 1) * 512], ps[:],
                                 AF.Identity, bias=b_in_c[:, mc:mc + 1])

        def evac_sg(mc, hv, ps):
            nc.scalar.activation(sg_f[:, mc, hv * 512:(hv + 1) * 512], ps[:],
                                 AF.Sigmoid, bias=b_a_c[:, mc:mc + 1])

        def evac_g(mc, hv, ps):
            # silu(v) = v * sigmoid(v), v = psum + b_g  (no Silu LUT on trn2)
            ps4 = ps[:].rearrange("p (b h w) -> p b h w", b=2, h=H, w=W)
            vt = scr.tile([128, 512], bf16, tag="gv", name=f"gv{mc}{hv}")
            nc.scalar.activation(vt[:], ps[:], AF.Sigmoid,
                                 bias=b_g_c[:, mc:mc + 1])
            vt4 = vt[:].rearrange("p (b h w) -> p b h w", b=2, h=H, w=W)
            for bb in range(2):
                nc.vector.scalar_tensor_tensor(
                    pad2(g_p, mc)[:, 2 * hv + bb, 2:2 + H, 2:2 + W],
                    ps4[:, bb], b_g_c[:, mc:mc + 1], vt4[:, bb],
                    op0=OP.add, op1=OP.mult)

        mm_c(evac_g, w_g_t)
        mm_c(evac_z, w_in_t)
        mm_c(evac_sg, w_a_t)

    # ---------------- E+F: Gt = DW5^T(g), P=z*Gt, fused Q-ladder ------------
    # Per chunk: 5x5 adjoint conv on PE; DVE stt forms P = z*Gt straight from
    # PSUM with a fused per-batch segment sum (accum_out), then 8 in-place
    # single-ALU ladder steps Q <- sigma*Q, each with fused per-batch segment
    # sums (all on DVE, overlapping the next chunk's conv on PE).  The rho
    # powers are folded into the St consumers (kv build / Horner weights), so
    # the ladder multiplies by plain sigma.
    taps5 = [(i, j) for i in range(5) for j in range(5)]
    if SMAX >= 5:
        for kc in range(NCC):
            for b in range(BL):
                nc.vector.tensor_reduce(
                    gbar_c[:, kc, b:b + 1], int2(g_p, kc)[:, b],
                    axis=AX.XY, op=OP.add)
            for hv in range(HV):
                ps = pp_mm.tile([128, 512], fp32, tag="mm", name=f"cgt{kc}{hv}")
                for ti, (i, j) in enumerate(taps5):
                    fi = (4 - i) * 5 + (4 - j)          # flipped kernel index
                    rhs = pad2(g_p, kc)[:, 2 * hv:2 * hv + 2, i:i + H, j:j + W]
                    nc.tensor.matmul(
                        ps[:], dsp_t[:, fi, kc, :], rhs,
                        start=(ti == 0), stop=(ti == 24))
                for bb in range(2):
                    b = 2 * hv + bb
                    nc.vector.scalar_tensor_tensor(
                        u_f[:, kc, b * HWN:(b + 1) * HWN],
                        z_f[:, kc, b * HWN:(b + 1) * HWN], 0.0,
                        ps[:, bb * HWN:(bb + 1) * HWN],
                        op0=OP.bypass, op1=OP.mult,
                        accum_out=s0_c[:, kc, b:b + 1])
            if SMAX >= 6:
                for t in range(T):
                    for b in range(BL):
                        nc.vector.scalar_tensor_tensor(
                            u_f[:, kc, b * HWN:(b + 1) * HWN],
                            u_f[:, kc, b * HWN:(b + 1) * HWN], 0.0,
                            sg_f[:, kc, b * HWN:(b + 1) * HWN],
                            op0=OP.bypass, op1=OP.mult,
                            accum_out=st_all[:, kc, b, t:t + 1])

    # ---------------- G: gate MLP + softmax ----------------
    if SMAX >= 7:
        inv = 1.0 / float(HWN)
        for kc in range(NCC):
            # s0gb = (S0 + b_sp*gbar) / HW
            nc.vector.scalar_tensor_tensor(
                s0gb[:, kc, :], gbar_c[:, kc, :], b_sp_c[:, kc:kc + 1],
                s0_c[:, kc, :], op0=OP.mult, op1=OP.add)
            nc.vector.tensor_scalar(
                s0gb[:, kc, :], s0gb[:, kc, :], inv, None, op0=OP.mult)
            for t in range(T):
                # fold the deferred rho^{t+1} of the sigma-only ladder in here
                nc.vector.scalar_tensor_tensor(
                    kv[:, kc, :, t], st_all[:, kc, :, t],
                    -inv * float(RHO ** (t + 1)), s0gb[:, kc, :],
                    op0=OP.mult, op1=OP.add)
        # q broadcast (zeros + per-partition scalar add)
        z32 = pool.tile([128, T], fp32, name="z32")
        nc.vector.memset(z32[:], 0.0)
        q_col = pool.tile([128, NCC, BL], fp32, name="q_col")
        for kc in range(NCC):
            nc.vector.tensor_scalar(
                q_col[:, kc, :], sview[:, 0, :, kc], 1.0 / float(HWN), None,
                op0=OP.mult)
            for b in range(BL):
                nc.vector.tensor_scalar(
                    qt[:, kc, b, :], z32[:], q_col[:, kc, b:b + 1], None, op0=OP.add)
        # k through W_out
        for mc in range(NCC):
            ps = pp_sm.tile([128, BL * T], fp32, tag="sm", name=f"kwm{mc}")
            for kc in range(NCC):
                nc.tensor.matmul(
                    ps[:], w_out_t[:, kc, mc * 128:(mc + 1) * 128],
                    kv[:, kc, :, :], start=(kc == 0), stop=(kc == NCC - 1))
            nc.scalar.activation(kw[:, mc, :], ps[:], AF.Identity,
                                 bias=b_out_c[:, mc:mc + 1])
        # gate hidden: gelu(v) ~= v*sigmoid(1.702 v)  (avoids Gelu table load)
        psg = pp_sm.tile([GH, BL * T], fp32, tag="sm", name="psg")
        for i in range(2 * NCC):
            rhs = qt[:, i, :, :] if i < NCC else kw[:, i - NCC, :]
            nc.tensor.matmul(psg[:], wg1_t[:, i, :], rhs,
                             start=(i == 0), stop=(i == 2 * NCC - 1))
        nc.scalar.activation(hgs[:], psg[:], AF.Sigmoid, bias=bg1_c[:],
                             scale=1.702)
        nc.vector.scalar_tensor_tensor(
            hg[:], psg[:], bg1_c[:], hgs[:], op0=OP.add, op1=OP.mult)
        psl = pp_sm.tile([1, BL * T], fp32, tag="sm", name="psl")
        nc.tensor.matmul(psl[:], wg2_t[:], hg[:], start=True, stop=True)
        nc.vector.scalar_tensor_tensor(
            logits[:], psl[:], bg2_c[:], prior_r[:], op0=OP.add, op1=OP.add)
        # softmax over t; logits are bounded (|mlp out| small + prior<=4) so
        # no max-subtraction needed in fp32
        nc.scalar.activation(esh[:], logits[:], AF.Exp)
        nc.vector.tensor_reduce(
            se_r[:], esh[:].rearrange("p (b t) -> p b t", b=BL), axis=AX.X, op=OP.add)
        nc.vector.reciprocal(se_r[:], se_r[:])
        for b in range(BL):
            nc.vector.tensor_scalar(
                wneg[:, b * T:(b + 1) * T], esh[:, b * T:(b + 1) * T],
                se_r[:, b:b + 1], -1.0, op0=OP.mult, op1=OP.mult)
        # fold rho^{t+1} into the weights (Horner then needs only *sigma)
        nc.vector.tensor_tensor(wneg[:], wneg[:], rhopow_r[:], op=OP.mult)
        nc.gpsimd.partition_broadcast(wbc[:], wneg[:], channels=128)

    # ---------------- H+I: Horner W-field + DW5(F) + W_out, interleaved ------
    # acc = -W via acc <- (acc + v_t)*sigma with v_t = -w_t rho^{t+1};
    # then F = (1+acc)*z in one fused stt per batch.  Chunk kc's conv (PE)
    # starts while chunk kc+1 runs Horner.  The middle chunk's Horner runs on
    # the (otherwise idle) GpSimd engine via broadcast tensor_tensor ops,
    # concurrently with chunk 0 on DVE.
    def wcol(b, t, n=HWN):
        c = wbc[:, b * T + t:b * T + t + 1]
        return bass.AP(c.tensor, c.offset, [list(c.ap[0]), [0, n]])

    if SMAX >= 8:
        acc = u_f  # ladder buffer is dead after stage F
        xo_rhs = xpos  # reuse xpos tile as W_out rhs buffer
        GKC = 1  # chunk offloaded to GpSimd
        for b in range(BL):
            sl = slice(b * HWN, (b + 1) * HWN)
            nc.gpsimd.tensor_tensor(
                acc[:, GKC, sl], sg_f[:, GKC, sl], wcol(b, 7), op=OP.mult)
            for t in range(6, -1, -1):
                nc.gpsimd.tensor_tensor(
                    acc[:, GKC, sl], acc[:, GKC, sl], wcol(b, t), op=OP.add)
                nc.gpsimd.tensor_tensor(
                    acc[:, GKC, sl], acc[:, GKC, sl], sg_f[:, GKC, sl],
                    op=OP.mult)
        for kc in range(NCC):
            for b in range(BL):
                sl = slice(b * HWN, (b + 1) * HWN)
                if kc != GKC:
                    nc.vector.tensor_scalar(
                        acc[:, kc, sl], sg_f[:, kc, sl],
                        wbc[:, b * T + 7:b * T + 8], None, op0=OP.mult)
                    for t in range(6, -1, -1):
                        nc.vector.scalar_tensor_tensor(
                            acc[:, kc, sl], acc[:, kc, sl],
                            wbc[:, b * T + t:b * T + t + 1],
                            sg_f[:, kc, sl], op0=OP.add, op1=OP.mult)
                nc.vector.scalar_tensor_tensor(
                    int2(f_p, kc)[:, b], dense(acc, kc)[:, b], 1.0,
                    dense(z_f, kc)[:, b], op0=OP.add, op1=OP.mult)
            if SMAX >= 9:
                for hv in range(HV):
                    ps = pp_mm.tile([128, 512], fp32, tag="mm", name=f"cf{kc}{hv}")
                    for ti, (i, j) in enumerate(taps5):
                        rhs = pad2(f_p, kc)[:, 2 * hv:2 * hv + 2, i:i + H, j:j + W]
                        nc.tensor.matmul(
                            ps[:], dsp_t[:, ti, kc, :], rhs,
                            start=(ti == 0), stop=(ti == 24))
                    ps4 = ps[:].rearrange("p (b h w) -> p b h w", b=2, h=H, w=W)
                    for bb in range(2):
                        b = 2 * hv + bb
                        nc.vector.scalar_tensor_tensor(
                            dense(xo_rhs, kc)[:, b], ps4[:, bb],
                            b_sp_c[:, kc:kc + 1],
                            int2(g_p, kc)[:, b],
                            op0=OP.add, op1=OP.mult)
        if SMAX >= 9:
            for mc in range(NCC):
                for hv in range(HV):
                    ps = pp_mm.tile([128, 512], fp32, tag="mm", name=f"wo{mc}{hv}")
                    for kc in range(NCC):
                        nc.tensor.matmul(
                            ps[:], w_out_t[:, kc, mc * 128:(mc + 1) * 128],
                            xo_rhs[:, kc, hv * 512:(hv + 1) * 512],
                            start=(kc == 0), stop=(kc == NCC - 1))
                    nc.vector.scalar_tensor_tensor(
                        out1[:, mc, hv * 512:(hv + 1) * 512],
                        ps[:], b_out_c[:, mc:mc + 1],
                        x_cm[:, mc, hv * 512:(hv + 1) * 512],
                        op0=OP.add, op1=OP.add)

    # ---------------- J: LN2 ----------------
    # Per-token stats land directly in [128, 8] token-partition layout via
    # data-stationary matmuls against ones/C, so the rstd chain runs on 128
    # partitions (the old [1,1024] chain cost ~15us serial).
    if SMAX >= 10:
        o1b = xpos  # reuse again: bf16 copy of out1
        for kc in range(NCC):
            nc.scalar.copy(o1b[:, kc, :], out1[:, kc, :])
            nc.vector.tensor_tensor(u_f[:, kc, :], o1b[:, kc, :], o1b[:, kc, :],
                                    op=OP.mult)   # squares into u_f
        psT = pp_sm.tile([128, 16], fp32, tag="sm", name="psT")
        for i in range(NTOK // 128):
            for kc in range(NCC):
                nc.tensor.matmul(psT[:, i:i + 1],
                                 o1b[:, kc, i * 128:(i + 1) * 128], ones_c[:],
                                 start=(kc == 0), stop=(kc == NCC - 1))
            for kc in range(NCC):
                nc.tensor.matmul(psT[:, 8 + i:9 + i],
                                 u_f[:, kc, i * 128:(i + 1) * 128], ones_c[:],
                                 start=(kc == 0), stop=(kc == NCC - 1))
        nc.vector.tensor_copy(ln2m[:], psT[:, 0:8])
        nc.vector.tensor_tensor(ln2v[:], ln2m[:], ln2m[:], op=OP.mult)
        nc.vector.tensor_tensor(ln2v[:], psT[:, 8:16], ln2v[:], op=OP.subtract)
        nc.vector.tensor_scalar(ln2v[:], ln2v[:], EPS, None, op0=OP.add)
        nc.scalar.sqrt(ln2v[:], ln2v[:])
        nc.vector.reciprocal(ln2v[:], ln2v[:])          # rstd [128, 8]
        nc.vector.tensor_copy(ln2b[:, 0:8], ln2v[:])
        nc.vector.scalar_tensor_tensor(
            ln2b[:, 8:16], ln2m[:], -1.0, ln2v[:], op0=OP.mult, op1=OP.mult)
        psb = pp_tr.tile([16, 128], fp32, tag="tr", name="psb")
        nc.tensor.matmul(psb[:], ln2b[:], ident[:], start=True, stop=True)
        nc.scalar.copy(rsb[:], psb[:])
        # stg is dead after stage A; borrow one partition row as the
        # [1, 2048] staging row for the per-token LN2 scale/shift
        rsrow = stg[:].rearrange("p a b -> p (a b)")[0:1, 0:2048]
        nc.sync.dma_start(
            rsrow.rearrange("o (si p) -> o si p", si=16), rsb[:])
        rhsS = rsrow[:, 0:1024]
        rhsM = rsrow[:, 1024:2048]
        yn = z_f  # reuse z tile as yn
        for kc in range(NCC):
            for hv in range(HV):
                psS = pp_mm.tile([128, 512], fp32, tag="mm", name=f"lnS{kc}{hv}")
                nc.tensor.matmul(psS[:], g2_t[0:1, kc, :],
                                 rhsS[:, hv * 512:(hv + 1) * 512],
                                 start=True, stop=True)
                psB = pp_mm.tile([128, 512], fp32, tag="mm", name=f"lnB{kc}{hv}")
                nc.tensor.matmul(psB[:], g2_t[0:1, kc, :],
                                 rhsM[:, hv * 512:(hv + 1) * 512],
                                 start=True, stop=True)
                nc.vector.tensor_tensor(
                    yn[:, kc, hv * 512:(hv + 1) * 512],
                    o1b[:, kc, hv * 512:(hv + 1) * 512], psS[:], op=OP.mult)
                nc.vector.scalar_tensor_tensor(
                    yn[:, kc, hv * 512:(hv + 1) * 512],
                    yn[:, kc, hv * 512:(hv + 1) * 512], be2_c[:, kc:kc + 1],
                    psB[:], op0=OP.add, op1=OP.add)

    # ---------------- K: MLP ----------------
    if SMAX >= 11:
        for jc in range(NHC):
            for hv in range(HV):
                ps = pp_mm.tile([128, 512], fp32, tag="mm", name=f"w1_{jc}{hv}")
                for kc in range(NCC):
                    nc.tensor.matmul(
                        ps[:], w1_t[:, kc, jc * 128:(jc + 1) * 128],
                        yn[:, kc, hv * 512:(hv + 1) * 512],
                        start=(kc == 0), stop=(kc == NCC - 1))
                ps4 = ps[:].rearrange("p (b h w) -> p b h w", b=2, h=H, w=W)
                for bb in range(2):
                    # DVE evac (fp8 write) keeps the Scalar engine free for
                    # the dwconv gelu evacs
                    nc.vector.tensor_scalar(
                        pad1(h1p, jc)[:, 2 * hv + bb, 1:1 + H, 1:1 + W],
                        ps4[:, bb], b1_c[:, jc:jc + 1], None, op0=OP.add)
        # depthwise 3x3 on HID channels: fp8e4 DoubleRow, two diagonal taps
        # contracted per pass (host scales the kernel by DW8S; the gelu evac
        # compensates via its activation scale).
        PAIRS = [((0, 0), (0, 1)), ((0, 2), (1, 0)),
                 ((1, 1), (1, 2)), ((2, 0), (2, 1))]
        SINGLE = (2, 2)
        DR = mybir.MatmulPerfMode.DoubleRow
        for jc in range(NHC):
            for b in range(BL):
                psw = pp_mm.tile([128, 512], fp32, tag="mm", name=f"cdw{jc}{b}")
                ps = psw[:, 0:256]
                rhs1 = pad1(h1p, jc)[:, b, SINGLE[0]:SINGLE[0] + H,
                                     SINGLE[1]:SINGLE[1] + W]
                nc.tensor.matmul(ps[:], ddw1_t[:, jc, :], rhs1,
                                 start=True, stop=True, skip_group_check=True)
                for pi, ((i0, j0), (i1, j1)) in enumerate(PAIRS):
                    base = pad1(h1p, jc)[:, b, i0:i0 + H, j0:j0 + W]
                    delta = (i1 - i0) * W1P + (j1 - j0)
                    rhs = bass.AP(base.tensor, base.offset,
                                  [list(base.ap[0]), [delta, 2],
                                   list(base.ap[1]), list(base.ap[2])])
                    nc.tensor.matmul(ps[:], ddw8_t[:, pi, jc, :, :], rhs,
                                     start=False, stop=(pi == 3), perf_mode=DR,
                                     skip_group_check=True)
                nc.scalar.activation(
                    pad1(h1p, jc)[:, b, 1:1 + H, 1:1 + W],
                    ps[:].rearrange("p (h w) -> p h w", h=H),
                    AF.Gelu_apprx_tanh, bias=bdw_c[:, jc:jc + 1],
                    scale=1.0 / DW8S)
        # W2 + output transpose/store, interleaved per token-half so the
        # first half's store overlaps the second half's W2 matmuls
        oh = sg_f   # dead by stage L, reuse
        ol = u_f
        out_s = pool.tile([128, NTOK // 128, C], fp32, name="out_s")
        out_dv = out_d[:].rearrange("(i p) c -> p i c", p=128)
        for hv in range(HV):
            sl = slice(hv * 512, (hv + 1) * 512)
            for mc in range(NCC):
                ps = pp_mm.tile([128, 512], fp32, tag="mm", name=f"w2_{mc}{hv}")
                for jc in range(NHC):
                    nc.tensor.matmul(
                        ps[:], w2_t[:, jc, mc * 128:(mc + 1) * 128],
                        int1(h1p, jc)[:, 2 * hv:2 * hv + 2],
                        start=(jc == 0), stop=(jc == NHC - 1))
                nc.vector.scalar_tensor_tensor(
                    x_cm[:, mc, sl], ps[:], b2_c[:, mc:mc + 1],
                    out1[:, mc, sl], op0=OP.add, op1=OP.add)
                nc.scalar.copy(oh[:, mc, sl], x_cm[:, mc, sl])
                nc.vector.scalar_tensor_tensor(
                    ol[:, mc, sl], oh[:, mc, sl], -1.0, x_cm[:, mc, sl],
                    op0=OP.mult, op1=OP.add)
            for i in range(hv * 4, hv * 4 + 4):
                for mc in range(NCC):
                    pt = pp_tr.tile([128, 128], fp32, tag="tr",
                                    name=f"tro{i}_{mc}")
                    nc.tensor.matmul(pt[:], oh[:, mc, i * 128:(i + 1) * 128],
                                     ident[:], start=True, stop=False)
                    nc.tensor.matmul(pt[:], ol[:, mc, i * 128:(i + 1) * 128],
                                     ident[:], start=False, stop=True)
                    nc.scalar.copy(out_s[:, i, mc * 128:(mc + 1) * 128], pt[:])
                nc.sync.dma_start(out_dv[:, i:i + 1], out_s[:, i:i + 1])

    ctx.close()


# ------------------------------------------------------------------
# host side
# ------------------------------------------------------------------

def _diagify(k2d, nchunks):
    """k2d: (KH, KW, 1, Cn) -> (KH*KW, nchunks, 128, 128) bf16 diagonals."""
    kh, kw = k2d.shape[0], k2d.shape[1]
    out = np.zeros((kh * kw, nchunks, 128, 128), dtype=BF16)
    idx = np.arange(128)
    for t in range(kh * kw):
        vals = k2d[t // kw, t % kw, 0].astype(np.float32)
        for c in range(nchunks):
            out[t, c, idx, idx] = vals[c * 128:(c + 1) * 128].astype(BF16)
    return out


def _prep_shared(w):
    """Build the shared (weight) input map from the raw input dict."""
    f32 = np.float32
    m = {}
    def pm(a):  # [k,128,...] -> [128,k,...] contiguous
        return np.ascontiguousarray(np.moveaxis(a, 1, 0))

    m["w_in"] = pm(w["W_in"].astype(f32).reshape(NCC, 128, C)).astype(BF16)
    m["w_a"] = pm(w["W_a"].astype(f32).reshape(NCC, 128, C)).astype(BF16)
    m["w_g"] = pm(w["W_g"].astype(f32).reshape(NCC, 128, C)).astype(BF16)
    m["w_out"] = pm(w["W_out"].astype(f32).reshape(NCC, 128, C)).astype(BF16)
    m["w1"] = pm(w["W1"].astype(f32).reshape(NCC, 128, HID)).astype(BF16)
    m["w2"] = pm(w["W2"].astype(f32).reshape(NHC, 128, C)).astype(BF16)
    m["wg1"] = pm(w["Wg1"].astype(f32).reshape(2 * NCC, 128, GH)).astype(BF16)
    m["wg2"] = w["Wg2"].astype(f32).reshape(GH, 1).astype(BF16)
    m["dpos"] = np.ascontiguousarray(
        _diagify(np.asarray(w["w_pos"]), NCC).transpose(2, 0, 1, 3))
    m["dsp"] = np.ascontiguousarray(
        _diagify(np.asarray(w["k_sp"]), NCC).transpose(2, 0, 1, 3))
    # fp8 DoubleRow tap-pair diagonals for the MLP depthwise conv, scaled by
    # DW8S (compensated in the gelu evac)
    wdw = np.asarray(w["wdw"], f32) * DW8S     # (3,3,1,HID)
    idx = np.arange(128)
    pairs = [((0, 0), (0, 1)), ((0, 2), (1, 0)),
             ((1, 1), (1, 2)), ((2, 0), (2, 1))]
    ddw8 = np.zeros((128, 4, NHC, 2, 128), dtype=FP8)
    for pi, (ta, tb) in enumerate(pairs):
        for k, t in enumerate((ta, tb)):
            vals = wdw[t[0], t[1], 0].reshape(NHC, 128)
            for jc in range(NHC):
                ddw8[idx, pi, jc, k, idx] = vals[jc].astype(FP8)
    ddw1 = np.zeros((128, NHC, 128), dtype=FP8)
    vals = wdw[2, 2, 0].reshape(NHC, 128)
    for jc in range(NHC):
        ddw1[idx, jc, idx] = vals[jc].astype(FP8)
    m["ddw8"] = ddw8
    m["ddw1"] = ddw1
    for src, dst, n in [("b_in", "b_in", NCC), ("b_a", "b_a", NCC),
                        ("b_g", "b_g", NCC), ("b_sp", "b_sp", NCC),
                        ("b_out", "b_out", NCC), ("b2", "b2", NCC),
                        ("gamma1", "gamma1", NCC), ("beta1", "beta1", NCC),
                        ("b1", "b1", NHC), ("bdw", "bdw", NHC)]:
        m[dst] = np.ascontiguousarray(np.asarray(w[src], f32).reshape(n, 128).T)
    m["b_pos"] = np.ascontiguousarray(
        np.asarray(w["b_pos"], f32).reshape(NCC, 128).T)
    m["g2r"] = np.asarray(w["gamma2"], f32).reshape(1, NCC, 128).astype(BF16)
    m["be2"] = np.ascontiguousarray(
        np.asarray(w["beta2"], f32).reshape(NCC, 128).T)
    m["bg1"] = np.asarray(w["bg1"], f32).reshape(GH, 1)
    m["bg2"] = np.asarray(w["bg2"], f32).reshape(1, 1)
    prior = np.zeros((T,), f32)
    prior[-1] = 4.0
    m["prior"] = np.tile(prior, BL)[None, :]
    rp = RHO ** (np.arange(1, T + 1, dtype=f32))
    m["rhopow"] = np.tile(rp, BL)[None, :].astype(f32)
    return m


TRACE = False       # set True (e.g. from test.py) to capture an NTFF profile
LAST_RES = None


def kernel(**inputs):
    global _PROG, LAST_RES
    from concourse.bass_utils import run_bass_kernel_spmd

    if _PROG is None:
        _PROG = _build_program()
    nc = _PROG

    shared = _prep_shared(inputs)
    x = np.asarray(inputs["x"], np.float32)
    in_maps = []
    for i in range(NCORES):
        im = dict(shared)
        xs = np.ascontiguousarray(x[i * BL:(i + 1) * BL].reshape(NTOK, C))
        xhi = xs.astype(BF16)
        im["x_hi"] = xhi
        im["x_lo"] = (xs - xhi.astype(np.float32)).astype(BF16)
        in_maps.append(im)

    res = run_bass_kernel_spmd(nc, in_maps, core_ids=list(range(NCORES)),
                               trace=TRACE)
    LAST_RES = res
    out = np.concatenate(
        [r["out"].reshape(BL, H, W, C) for r in res.results], axis=0)
    return out


# revision 51
# speedup vs baseline: 1.0860x; 1.0860x over previous
"""Trainium2 Bass kernel for nn_CSSMSHViT_60043642798201.

Strategy
--------
The reference repeats the input image over a time axis T=8 and runs a gated
scalar recurrence over T.  Because the input is constant over T the whole
temporal structure collapses algebraically:

    h_t = (1 - a^{t+1}) z          (closed form of the scan)

so the per-timestep fields are never materialised.  The kernel computes

  LN1 (global per-batch) -> +3x3 depthwise pos conv -> z/sigma/g projections
  -> power ladder u_t = a^t z with fused per-batch reductions St = sum(u_t*Gt)
     where Gt = DW5^T(g)  (adjoint trick: mean(DW5(u)*g) = mean(u*DW5^T(g)))
  -> tiny gate MLP -> softmax weights w
  -> F = z - sum_t w_t u_{t+1};  x_out = (DW5(F)+b_sp)*g @ W_out + b_out
  -> out1 = x + x_out -> LN2 -> MLP with 3x3 depthwise conv -> out.

Sharding: pure data-parallel over batch (32 = 8 cores x 4), no collectives.

Perf structure (v2):
  * the Q-ladder runs as fused tensor_tensor_reduce ops (multiply + per-batch
    segment sum in one DVE instr, reading conv PSUM directly) interleaved
    chunk-by-chunk with the 5x5 conv so the scan hides under PE conv time;
  * the Horner recurrence for W = sum_t w_t a^{t+1} folds the rho powers into
    the softmax weights so each step is one per-batch stt, and the final
    F = z*(1-W) is a single fused stt per batch;
  * MLP depthwise-conv diagonal weights are precomputed on host and DMAed;
  * LN2 stats are transposed into [128,8] via tiny data-stationary matmuls
    (avoids a 6.5us single-partition reciprocal);
  * LN1 squares via fused ttr; gate-MLP gelu via sigmoid approx (saves
    activation table loads).
"""

import numpy as np
import ml_dtypes

BF16 = ml_dtypes.bfloat16
FP8 = ml_dtypes.float8_e4m3

# problem constants
B, T, H, W, C = 32, 8, 16, 16, 384
KS = 5
HID = 4 * C
GH = max(C // 4, 8)
RHO = 0.999
EPS = 1e-6

NCORES = 8
BL = B // NCORES            # batches per core = 4
HWN = H * W                 # 256 tokens per image
NTOK = BL * HWN             # 1024 tokens per core
NCC = C // 128              # 3 channel chunks
NHC = HID // 128            # 12 hidden chunks

DW8S = 32.0                 # fp8 scale folded into the MLP dwconv kernel

# padded geometries (channel-major fields, free layout (b, hp, wp))
H1, W1P = 18, 18            # pad-1 buffers (3x3 convs)
F1 = BL * H1 * W1P
H2, W2P = 20, 20            # pad-2 buffers (5x5 convs)
F2 = BL * H2 * W2P

_PROG = None  # cached compiled program


def _build_program():
    import concourse.bass as bass
    import concourse.tile as tile
    from concourse import bacc, mybir

    fp32 = mybir.dt.float32
    bf16 = mybir.dt.bfloat16
    AF = mybir.ActivationFunctionType
    OP = mybir.AluOpType
    AX = mybir.AxisListType

    nc = bacc.Bacc("TRN2", target_bir_lowering=False)

    # ---------------- DRAM tensors ----------------
    d = {}
    d["x_hi"] = nc.dram_tensor("x_hi", [NTOK, C], bf16, kind="ExternalInput")
    d["x_lo"] = nc.dram_tensor("x_lo", [NTOK, C], bf16, kind="ExternalInput")
    # matmul weights, chunked [kchunks, 128, M] bf16
    d["w_in"] = nc.dram_tensor("w_in", [128, NCC, C], bf16, kind="ExternalInput")
    d["w_a"] = nc.dram_tensor("w_a", [128, NCC, C], bf16, kind="ExternalInput")
    d["w_g"] = nc.dram_tensor("w_g", [128, NCC, C], bf16, kind="ExternalInput")
    d["w_out"] = nc.dram_tensor("w_out", [128, NCC, C], bf16, kind="ExternalInput")
    d["w1"] = nc.dram_tensor("w1", [128, NCC, HID], bf16, kind="ExternalInput")
    d["w2"] = nc.dram_tensor("w2", [128, NHC, C], bf16, kind="ExternalInput")
    d["wg1"] = nc.dram_tensor("wg1", [128, 2 * NCC, GH], bf16, kind="ExternalInput")
    d["wg2"] = nc.dram_tensor("wg2", [GH, 1], bf16, kind="ExternalInput")
    # diagonalised depthwise kernels (partition-major, contiguous per partition)
    fp8 = mybir.dt.float8e4
    d["dpos"] = nc.dram_tensor("dpos", [128, 9, NCC, 128], bf16, kind="ExternalInput")
    d["dsp"] = nc.dram_tensor("dsp", [128, 25, NCC, 128], bf16, kind="ExternalInput")
    # MLP depthwise 3x3 kernel as fp8 diagonal tap-pairs (DoubleRow) + single
    d["ddw8"] = nc.dram_tensor("ddw8", [128, 4, NHC, 2, 128], fp8,
                               kind="ExternalInput")
    d["ddw1"] = nc.dram_tensor("ddw1", [128, NHC, 128], fp8, kind="ExternalInput")
    # per-channel vectors [128, nchunks] fp32
    for nm in ["b_in", "b_a", "b_g", "b_sp", "b_out", "b2", "gamma1", "beta1",
               "b_pos"]:
        d[nm] = nc.dram_tensor(nm, [128, NCC], fp32, kind="ExternalInput")
    d["b1"] = nc.dram_tensor("b1", [128, NHC], fp32, kind="ExternalInput")
    d["bdw"] = nc.dram_tensor("bdw", [128, NHC], fp32, kind="ExternalInput")
    d["g2r"] = nc.dram_tensor("g2r", [1, NCC, 128], bf16, kind="ExternalInput")
    d["be2"] = nc.dram_tensor("be2", [128, NCC], fp32, kind="ExternalInput")
    d["bg1"] = nc.dram_tensor("bg1", [GH, 1], fp32, kind="ExternalInput")
    d["bg2"] = nc.dram_tensor("bg2", [1, 1], fp32, kind="ExternalInput")
    d["prior"] = nc.dram_tensor("prior", [1, BL * T], fp32, kind="ExternalInput")
    d["rhopow"] = nc.dram_tensor("rhopow", [1, BL * T], fp32, kind="ExternalInput")
    d["rif"] = nc.dram_tensor("rif", [128, T], fp32, kind="ExternalInput")
    out_d = nc.dram_tensor("out", [NTOK, C], fp32, kind="ExternalOutput")

    with tile.TileContext(nc) as tc:
        _emit(nc, tc, d, out_d, mybir, bass, fp32, bf16, fp8, AF, OP, AX)

    nc.compile()
    return nc


def _emit(nc, tc, d, out_d, mybir, bass, fp32, bf16, fp8, AF, OP, AX):
    import os
    SMAX = int(os.environ.get("BASS_SMAX", "99"))
    from contextlib import ExitStack
    ctx = ExitStack()

    pool = ctx.enter_context(tc.tile_pool(name="persist", bufs=1))
    scr = ctx.enter_context(tc.tile_pool(name="scratch", bufs=2))
    pp_mm = ctx.enter_context(tc.tile_pool(name="pp_mm", bufs=5, space="PSUM"))
    pp_tr = ctx.enter_context(tc.tile_pool(name="pp_tr", bufs=2, space="PSUM"))
    pp_sm = ctx.enter_context(tc.tile_pool(name="pp_sm", bufs=1, space="PSUM"))

    # ---------------- persistent field tiles ----------------
    x_cm = pool.tile([128, NCC, NTOK], fp32, name="x_cm")          # also final out
    xn0p = pool.tile([128, NCC, F1], bf16, name="xn0p")            # padded LN1 out
    xpos = pool.tile([128, NCC, NTOK], bf16, name="xpos")
    z_f = pool.tile([128, NCC, NTOK], bf16, name="z_f")            # reused as yn
    sg_f = pool.tile([128, NCC, NTOK], bf16, name="sg_f")
    g_p = pool.tile([128, NCC, F2], bf16, name="g_p")              # padded silu gate
    u_f = pool.tile([128, NCC, NTOK], bf16, name="u_f")            # ladder / acc / sq
    f_p = pool.tile([128, NCC, F2], bf16, name="f_p")              # padded F field
    out1 = pool.tile([128, NCC, NTOK], fp32, name="out1")          # also out staging
    h1p = pool.tile([128, NHC, F1], fp8, name="h1p")               # padded MLP hidden

    # weights
    w_in_t = pool.tile([128, NCC, C], bf16, name="w_in_t")
    w_a_t = pool.tile([128, NCC, C], bf16, name="w_a_t")
    w_g_t = pool.tile([128, NCC, C], bf16, name="w_g_t")
    w_out_t = pool.tile([128, NCC, C], bf16, name="w_out_t")
    w1_t = pool.tile([128, NCC, HID], bf16, name="w1_t")
    w2_t = pool.tile([128, NHC, C], bf16, name="w2_t")
    wg1_t = pool.tile([128, 2 * NCC, GH], bf16, name="wg1_t")
    wg2_t = pool.tile([GH, 1], bf16, name="wg2_t")
    dsp_t = pool.tile([128, 25, NCC, 128], bf16, name="dsp_t")     # resident 5x5 diags
    dpos_t = pool.tile([128, 9, NCC, 128], bf16, name="dpos_t")    # resident 3x3 diags
    ddw8_t = pool.tile([128, 4, NHC, 2, 128], fp8, name="ddw8_t")  # dw diag pairs
    ddw1_t = pool.tile([128, NHC, 128], fp8, name="ddw1_t")        # dw diag single

    # vectors
    b_in_c = pool.tile([128, NCC], fp32, name="b_in_c")
    b_a_c = pool.tile([128, NCC], fp32, name="b_a_c")
    b_g_c = pool.tile([128, NCC], fp32, name="b_g_c")
    b_sp_c = pool.tile([128, NCC], fp32, name="b_sp_c")
    b_pos_c = pool.tile([128, NCC], fp32, name="b_pos_c")
    b_out_c = pool.tile([128, NCC], fp32, name="b_out_c")
    b2_c = pool.tile([128, NCC], fp32, name="b2_c")
    g1_c = pool.tile([128, NCC], fp32, name="g1_c")
    be1_c = pool.tile([128, NCC], fp32, name="be1_c")
    b1_c = pool.tile([128, NHC], fp32, name="b1_c")
    bdw_c = pool.tile([128, NHC], fp32, name="bdw_c")
    g2_t = pool.tile([1, NCC, 128], bf16, name="g2_t")
    be2_c = pool.tile([128, NCC], fp32, name="be2_c")
    bg1_c = pool.tile([GH, 1], fp32, name="bg1_c")
    bg2_c = pool.tile([1, 1], fp32, name="bg2_c")
    prior_r = pool.tile([1, BL * T], fp32, name="prior_r")
    rhopow_r = pool.tile([1, BL * T], fp32, name="rhopow_r")
    rif_c = pool.tile([128, T], fp32, name="rif_c")

    # small working tiles
    ident = pool.tile([128, 128], bf16, name="ident")
    ones_c = pool.tile([128, 1], bf16, name="ones_c")    # holds 1/C (LN2 stats)
    sums = pool.tile([128, 24], fp32, name="sums")       # stat*12 + b*3 + kc
    ar = pool.tile([128, 24], fp32, name="ar")
    tot = pool.tile([128, 2, BL], fp32, name="tot")
    m_col = pool.tile([128, BL], fp32, name="m_col")
    e2_col = pool.tile([128, BL], fp32, name="e2_col")
    var_col = pool.tile([128, BL], fp32, name="var_col")
    rstd_col = pool.tile([128, BL], fp32, name="rstd_col")
    sc_col = pool.tile([128, NCC, BL], fp32, name="sc_col")
    bi_col = pool.tile([128, NCC, BL], fp32, name="bi_col")
    tmp_col = pool.tile([128, BL], fp32, name="tmp_col")
    st_all = pool.tile([128, NCC, BL, T], fp32, name="st_all")
    s0_c = pool.tile([128, NCC, BL], fp32, name="s0_c")
    gbar_c = pool.tile([128, NCC, BL], fp32, name="gbar_c")
    s0gb = pool.tile([128, NCC, BL], fp32, name="s0gb")
    kv = pool.tile([128, NCC, BL, T], bf16, name="kv")
    qt = pool.tile([128, NCC, BL, T], bf16, name="qt")
    kw = pool.tile([128, NCC, BL * T], bf16, name="kw")
    hg = pool.tile([GH, BL * T], bf16, name="hg")
    hgs = pool.tile([GH, BL * T], bf16, name="hgs")
    logits = pool.tile([1, BL * T], fp32, name="logits")
    esh = pool.tile([1, BL * T], fp32, name="esh")
    se_r = pool.tile([1, BL], fp32, name="se_r")
    wneg = pool.tile([1, BL * T], fp32, name="wneg")
    wbc = pool.tile([128, BL * T], fp32, name="wbc")
    # LN2 small tiles
    ln2b = pool.tile([128, 16], bf16, name="ln2b")       # rstd8 | -mu*rstd8 (bf16)
    ln2m = pool.tile([128, 8], fp32, name="ln2m")        # mu
    ln2v = pool.tile([128, 8], fp32, name="ln2v")        # var -> rstd
    rsb = pool.tile([16, 128], bf16, name="rsb")

    # ---------------- loads (x first; then in consumption order) ----------------
    stg = pool.tile([128, NTOK // 128, 2 * C], bf16, name="stg")
    xhi_s = stg[:, :, 0:C]
    xlo_s = stg[:, :, C:2 * C]
    nc.sync.dma_start(
        xhi_s, d["x_hi"][:].rearrange("(i p) c -> p i c", p=128))
    nc.sync.dma_start(
        xlo_s, d["x_lo"][:].rearrange("(i p) c -> p i c", p=128))

    def ld(tile_ap, dram):
        nc.sync.dma_start(tile_ap[:], dram[:])

    for nm, t_ in [("gamma1", g1_c), ("beta1", be1_c), ("b_pos", b_pos_c),
                   ("b_in", b_in_c), ("b_a", b_a_c), ("b_g", b_g_c),
                   ("b_sp", b_sp_c), ("b_out", b_out_c), ("b2", b2_c)]:
        ld(t_, d[nm])
    ld(dpos_t, d["dpos"])
    ld(w_g_t, d["w_g"])
    ld(w_in_t, d["w_in"])
    ld(w_a_t, d["w_a"])
    ld(dsp_t, d["dsp"])
    ld(w_out_t, d["w_out"])
    ld(wg1_t, d["wg1"])
    nc.sync.dma_start(wg2_t[:], d["wg2"][:])
    nc.sync.dma_start(g2_t[:], d["g2r"][:])
    ld(be2_c, d["be2"])
    nc.sync.dma_start(bg1_c[:], d["bg1"][:])
    nc.sync.dma_start(bg2_c[:], d["bg2"][:])
    nc.sync.dma_start(prior_r[:], d["prior"][:])
    nc.sync.dma_start(rhopow_r[:], d["rhopow"][:])
    nc.sync.dma_start(rif_c[:], d["rif"][:])
    ld(w1_t, d["w1"])
    ld(ddw8_t, d["ddw8"])
    ld(ddw1_t, d["ddw1"])
    ld(w2_t, d["w2"])
    ld(b1_c, d["b1"])
    ld(bdw_c, d["bdw"])

    from concourse.masks import make_identity
    make_identity(nc, ident[:])
    nc.vector.memset(ones_c[:], 1.0 / float(C))

    # zero padded buffers (borders must stay zero); xn0p first (needed soonest)
    nc.gpsimd.memset(xn0p[:].rearrange("p a b -> p (a b)"), 0.0)
    nc.gpsimd.memset(g_p[:].rearrange("p a b -> p (a b)"), 0.0)
    nc.gpsimd.memset(f_p[:].rearrange("p a b -> p (a b)"), 0.0)
    nc.gpsimd.memset(h1p[:].rearrange("p a b -> p (a b)"), 0.0)

    # view helpers -------------------------------------------------
    def pad1(tile_, j):           # -> [128, BL, H1, W1P] for chunk j
        return tile_[:, j, :].rearrange("p (b h w) -> p b h w", b=BL, h=H1, w=W1P)

    def pad2(tile_, j):
        return tile_[:, j, :].rearrange("p (b h w) -> p b h w", b=BL, h=H2, w=W2P)

    def dense(tile_, j):          # -> [128, BL, H, W]
        return tile_[:, j, :].rearrange("p (b h w) -> p b h w", b=BL, h=H, w=W)

    def int1(tile_, j):           # pad1 interior
        return pad1(tile_, j)[:, :, 1:1 + H, 1:1 + W]

    def int2(tile_, j):
        return pad2(tile_, j)[:, :, 2:2 + H, 2:2 + W]

    HV = NTOK // 512              # 2 halves (2 batches each)

    # ---------------- A+B: transpose x on PE, LN1 stats interleaved ----------
    sview = sums[:].rearrange("p (s b k) -> p s b k", s=2, b=BL, k=NCC)
    for kc in range(NCC):
        for i in range(NTOK // 128):
            pt = pp_tr.tile([128, 128], fp32, tag="tr", name=f"trx{i}_{kc}")
            nc.tensor.matmul(pt[:], xhi_s[:, i, kc * 128:(kc + 1) * 128],
                             ident[:], start=True, stop=False)
            nc.tensor.matmul(pt[:], xlo_s[:, i, kc * 128:(kc + 1) * 128],
                             ident[:], start=False, stop=True)
            nc.scalar.copy(x_cm[:, kc, i * 128:(i + 1) * 128], pt[:])
        if SMAX >= 2:
            nc.vector.tensor_reduce(
                sview[:, 0, :, kc],
                x_cm[:, kc, :].rearrange("p (b n) -> p b n", b=BL),
                axis=AX.X, op=OP.add)
            for b in range(BL):
                s_sc = scr.tile([128, HWN], bf16, tag="sq_scr", name=f"sxx{kc}{b}")
                nc.vector.scalar_tensor_tensor(
                    s_sc[:], x_cm[:, kc, b * HWN:(b + 1) * HWN], 0.0,
                    x_cm[:, kc, b * HWN:(b + 1) * HWN],
                    op0=OP.bypass, op1=OP.mult,
                    accum_out=sview[:, 1, b, kc:kc + 1])

    # ---------------- B: LN1 stats finalize + apply ----------------
    if SMAX >= 2:
        import concourse.bass_isa as bass_isa
        RADD = bass_isa.ReduceOp.add
        nc.gpsimd.partition_all_reduce(ar[:], sums[:], channels=128, reduce_op=RADD)
        nc.vector.tensor_reduce(
            tot[:], ar[:].rearrange("p (s b k) -> p s b k", s=2, b=BL, k=NCC),
            axis=AX.X, op=OP.add)
        NB = float(HWN * C)
        nc.vector.tensor_scalar(m_col[:], tot[:, 0, :], 1.0 / NB, None, op0=OP.mult)
        nc.vector.tensor_scalar(e2_col[:], tot[:, 1, :], 1.0 / NB, None, op0=OP.mult)
        nc.vector.tensor_tensor(tmp_col[:], m_col[:], m_col[:], op=OP.mult)
        nc.vector.tensor_tensor(var_col[:], e2_col[:], tmp_col[:], op=OP.subtract)
        nc.vector.tensor_scalar(var_col[:], var_col[:], EPS, None, op0=OP.add)
        nc.scalar.sqrt(var_col[:], var_col[:])
        nc.vector.reciprocal(rstd_col[:], var_col[:])
        for kc in range(NCC):
            nc.vector.tensor_scalar(
                sc_col[:, kc, :], rstd_col[:], g1_c[:, kc:kc + 1], None, op0=OP.mult)
            nc.vector.tensor_tensor(tmp_col[:], m_col[:], sc_col[:, kc, :], op=OP.mult)
            nc.vector.tensor_scalar(
                bi_col[:, kc, :], tmp_col[:], be1_c[:, kc:kc + 1], -1.0,
                op0=OP.subtract, op1=OP.mult)
            for b in range(BL):
                nc.scalar.activation(
                    pad1(xn0p, kc)[:, b, 1:1 + H, 1:1 + W],
                    dense(x_cm, kc)[:, b],
                    AF.Identity,
                    bias=bi_col[:, kc, b:b + 1], scale=sc_col[:, kc, b:b + 1])
        # gate-MLP q input (broadcast over t) — built here, far off the
        # gate critical path
        z32 = pool.tile([128, T], fp32, name="z32")
        nc.vector.memset(z32[:], 0.0)
        q_col = pool.tile([128, NCC, BL], fp32, name="q_col")
        for kc in range(NCC):
            nc.vector.tensor_scalar(
                q_col[:, kc, :], sview[:, 0, :, kc], 1.0 / float(HWN), None,
                op0=OP.mult)
            for b in range(BL):
                nc.vector.tensor_scalar(
                    qt[:, kc, b, :], z32[:], q_col[:, kc, b:b + 1], None,
                    op0=OP.add)

    # ---------------- C: positional 3x3 conv -> xpos ----------------
    if SMAX >= 3:
        for kc in range(NCC):
            for hv in range(HV):
                ps = pp_mm.tile([128, 512], fp32, tag="mm", name=f"cpos{kc}{hv}")
                for ti, (i, j) in enumerate([(a, b) for a in range(3) for b in range(3)]):
                    rhs = pad1(xn0p, kc)[:, 2 * hv:2 * hv + 2, i:i + H, j:j + W]
                    nc.tensor.matmul(
                        ps[:], dpos_t[:, ti, kc, :], rhs,
                        start=(ti == 0), stop=(ti == 8))
                ps4 = ps[:].rearrange("p (b h w) -> p b h w", b=2, h=H, w=W)
                for bb in range(2):
                    b = 2 * hv + bb
                    nc.vector.scalar_tensor_tensor(
                        dense(xpos, kc)[:, b], ps4[:, bb],
                        b_pos_c[:, kc:kc + 1],
                        int1(xn0p, kc)[:, b],
                        op0=OP.add, op1=OP.add)

    # ---------------- D: z / sigma / g projections ----------------
    if SMAX >= 4:
        def mm_c(dst_evac, w_t, hv_count=HV):
            for mc in range(NCC):
                for hv in range(hv_count):
                    ps = pp_mm.tile([128, 512], fp32, tag="mm",
                                    name=f"mmc_{id(w_t)}_{mc}_{hv}")
                    for kc in range(NCC):
                        nc.tensor.matmul(
                            ps[:], w_t[:, kc, mc * 128:(mc + 1) * 128],
                            xpos[:, kc, hv * 512:(hv + 1) * 512],
                            start=(kc == 0), stop=(kc == NCC - 1))
                    dst_evac(mc, hv, ps)

        def evac_z(mc, hv, ps):
            nc.scalar.activation(z_f[:, mc, hv * 512:(hv + 1) * 512], ps[:],
                                 AF.Identity, bias=b_in_c[:, mc:mc + 1])

        def evac_sg(mc, hv, ps):
            nc.scalar.activation(sg_f[:, mc, hv * 512:(hv + 1) * 512], ps[:],
                                 AF.Sigmoid, bias=b_a_c[:, mc:mc + 1])

        def evac_g(mc, hv, ps):
            # silu(v) = v * sigmoid(v), v = psum + b_g  (no Silu LUT on trn2)
            ps4 = ps[:].rearrange("p (b h w) -> p b h w", b=2, h=H, w=W)
            vt = scr.tile([128, 512], bf16, tag="gv", name=f"gv{mc}{hv}")
            nc.scalar.activation(vt[:], ps[:], AF.Sigmoid,
                                 bias=b_g_c[:, mc:mc + 1])
            vt4 = vt[:].rearrange("p (b h w) -> p b h w", b=2, h=H, w=W)
            for bb in range(2):
                nc.vector.scalar_tensor_tensor(
                    pad2(g_p, mc)[:, 2 * hv + bb, 2:2 + H, 2:2 + W],
                    ps4[:, bb], b_g_c[:, mc:mc + 1], vt4[:, bb],
                    op0=OP.add, op1=OP.mult)

        mm_c(evac_g, w_g_t)
        mm_c(evac_z, w_in_t)
        mm_c(evac_sg, w_a_t)

    # ---------------- E+F: Gt = DW5^T(g), P=z*Gt, fused Q-ladder ------------
    # Per chunk: 5x5 adjoint conv on PE; DVE stt forms P = z*Gt straight from
    # PSUM with a fused per-batch segment sum (accum_out), then 8 in-place
    # single-ALU ladder steps Q <- sigma*Q, each with fused per-batch segment
    # sums (all on DVE, overlapping the next chunk's conv on PE).  The rho
    # powers are folded into the St consumers (kv build / Horner weights), so
    # the ladder multiplies by plain sigma.
    taps5 = [(i, j) for i in range(5) for j in range(5)]
    if SMAX >= 5:
        for kc in range(NCC):
            for b in range(BL):
                nc.vector.tensor_reduce(
                    gbar_c[:, kc, b:b + 1], int2(g_p, kc)[:, b],
                    axis=AX.XY, op=OP.add)
            for hv in range(HV):
                ps = pp_mm.tile([128, 512], fp32, tag="mm", name=f"cgt{kc}{hv}")
                for ti, (i, j) in enumerate(taps5):
                    fi = (4 - i) * 5 + (4 - j)          # flipped kernel index
                    rhs = pad2(g_p, kc)[:, 2 * hv:2 * hv + 2, i:i + H, j:j + W]
                    nc.tensor.matmul(
                        ps[:], dsp_t[:, fi, kc, :], rhs,
                        start=(ti == 0), stop=(ti == 24))
                for bb in range(2):
                    b = 2 * hv + bb
                    nc.vector.scalar_tensor_tensor(
                        u_f[:, kc, b * HWN:(b + 1) * HWN],
                        z_f[:, kc, b * HWN:(b + 1) * HWN], 0.0,
                        ps[:, bb * HWN:(bb + 1) * HWN],
                        op0=OP.bypass, op1=OP.mult,
                        accum_out=s0_c[:, kc, b:b + 1])
                if SMAX >= 6:
                    # ladder chains start per conv-half, not per chunk
                    for bb in range(2):
                        b = 2 * hv + bb
                        for t in range(T):
                            nc.vector.scalar_tensor_tensor(
                                u_f[:, kc, b * HWN:(b + 1) * HWN],
                                u_f[:, kc, b * HWN:(b + 1) * HWN], 0.0,
                                sg_f[:, kc, b * HWN:(b + 1) * HWN],
                                op0=OP.bypass, op1=OP.mult,
                                accum_out=st_all[:, kc, b, t:t + 1])
            # s0gb = (S0 + b_sp*gbar) / HW  (off the gate critical path)
            nc.vector.scalar_tensor_tensor(
                s0gb[:, kc, :], gbar_c[:, kc, :], b_sp_c[:, kc:kc + 1],
                s0_c[:, kc, :], op0=OP.mult, op1=OP.add)
            nc.vector.tensor_scalar(
                s0gb[:, kc, :], s0gb[:, kc, :], 1.0 / float(HWN), None,
                op0=OP.mult)

    # ---------------- G: gate MLP + softmax ----------------
    if SMAX >= 7:
        inv = 1.0 / float(HWN)
        for kc in range(NCC):
            # kv = s0gb - inv*rho^{t+1}*St, via stride-0 broadcast operands
            # (rif carries -inv*rho^{t+1}; the deferred rho of the sigma-only
            # ladder folds in here)
            rifb = bass.AP(rif_c[:].tensor, rif_c[:].offset,
                           [list(rif_c[:].ap[0]), [0, BL], [1, T]])
            s0gb3 = s0gb[:, kc, :]
            s0gbb = bass.AP(s0gb3.tensor, s0gb3.offset,
                            [list(s0gb3.ap[0]), list(s0gb3.ap[1]), [0, T]])
            nc.vector.tensor_tensor(
                kv[:, kc, :, :], st_all[:, kc, :, :], rifb, op=OP.mult)
            nc.vector.tensor_tensor(
                kv[:, kc, :, :], kv[:, kc, :, :], s0gbb, op=OP.add)
        # k through W_out
        for mc in range(NCC):
            ps = pp_sm.tile([128, BL * T], fp32, tag="sm", name=f"kwm{mc}")
            for kc in range(NCC):
                nc.tensor.matmul(
                    ps[:], w_out_t[:, kc, mc * 128:(mc + 1) * 128],
                    kv[:, kc, :, :], start=(kc == 0), stop=(kc == NCC - 1))
            nc.scalar.activation(kw[:, mc, :], ps[:], AF.Identity,
                                 bias=b_out_c[:, mc:mc + 1])
        # gate hidden: gelu(v) ~= v*sigmoid(1.702 v)  (avoids Gelu table load)
        psg = pp_sm.tile([GH, BL * T], fp32, tag="sm", name="psg")
        for i in range(2 * NCC):
            rhs = qt[:, i, :, :] if i < NCC else kw[:, i - NCC, :]
            nc.tensor.matmul(psg[:], wg1_t[:, i, :], rhs,
                             start=(i == 0), stop=(i == 2 * NCC - 1))
        nc.scalar.activation(hgs[:], psg[:], AF.Sigmoid, bias=bg1_c[:],
                             scale=1.702)
        nc.vector.scalar_tensor_tensor(
            hg[:], psg[:], bg1_c[:], hgs[:], op0=OP.add, op1=OP.mult)
        psl = pp_sm.tile([1, BL * T], fp32, tag="sm", name="psl")
        nc.tensor.matmul(psl[:], wg2_t[:], hg[:], start=True, stop=True)
        nc.vector.scalar_tensor_tensor(
            logits[:], psl[:], bg2_c[:], prior_r[:], op0=OP.add, op1=OP.add)
        # softmax over t; logits are bounded (|mlp out| small + prior<=4) so
        # no max-subtraction needed in fp32
        nc.scalar.activation(esh[:], logits[:], AF.Exp)
        nc.vector.tensor_reduce(
            se_r[:], esh[:].rearrange("p (b t) -> p b t", b=BL), axis=AX.X, op=OP.add)
        nc.vector.reciprocal(se_r[:], se_r[:])
        for b in range(BL):
            nc.vector.tensor_scalar(
                wneg[:, b * T:(b + 1) * T], esh[:, b * T:(b + 1) * T],
                se_r[:, b:b + 1], -1.0, op0=OP.mult, op1=OP.mult)
        # fold rho^{t+1} into the weights (Horner then needs only *sigma)
        nc.vector.tensor_tensor(wneg[:], wneg[:], rhopow_r[:], op=OP.mult)
        nc.gpsimd.partition_broadcast(wbc[:], wneg[:], channels=128)

    # ---------------- H+I: Horner W-field + DW5(F) + W_out, interleaved ------
    # acc = -W via acc <- (acc + v_t)*sigma with v_t = -w_t rho^{t+1};
    # then F = (1+acc)*z in one fused stt per batch.  Chunk kc's conv (PE)
    # starts while chunk kc+1 runs Horner.  The middle chunk's Horner runs on
    # the (otherwise idle) GpSimd engine via broadcast tensor_tensor ops,
    # concurrently with chunk 0 on DVE.
    def wcol(b, t, n=HWN):
        c = wbc[:, b * T + t:b * T + t + 1]
        return bass.AP(c.tensor, c.offset, [list(c.ap[0]), [0, n]])

    if SMAX >= 8:
        acc = u_f  # ladder buffer is dead after stage F
        xo_rhs = xpos  # reuse xpos tile as W_out rhs buffer
        for kc in range(NCC):
            for hv in range(HV):
                for bb in range(2):
                    b = 2 * hv + bb
                    sl = slice(b * HWN, (b + 1) * HWN)
                    nc.vector.tensor_scalar(
                        acc[:, kc, sl], sg_f[:, kc, sl],
                        wbc[:, b * T + 7:b * T + 8], None, op0=OP.mult)
                    for t in range(6, -1, -1):
                        nc.vector.scalar_tensor_tensor(
                            acc[:, kc, sl], acc[:, kc, sl],
                            wbc[:, b * T + t:b * T + t + 1],
                            sg_f[:, kc, sl], op0=OP.add, op1=OP.mult)
                    nc.vector.scalar_tensor_tensor(
                        int2(f_p, kc)[:, b], dense(acc, kc)[:, b], 1.0,
                        dense(z_f, kc)[:, b], op0=OP.add, op1=OP.mult)
                if SMAX >= 9:
                    # conv of this token-half right after its two batches
                    ps = pp_mm.tile([128, 512], fp32, tag="mm", name=f"cf{kc}{hv}")
                    for ti, (i, j) in enumerate(taps5):
                        rhs = pad2(f_p, kc)[:, 2 * hv:2 * hv + 2, i:i + H, j:j + W]
                        nc.tensor.matmul(
                            ps[:], dsp_t[:, ti, kc, :], rhs,
                            start=(ti == 0), stop=(ti == 24))
                    ps4 = ps[:].rearrange("p (b h w) -> p b h w", b=2, h=H, w=W)
                    for bb in range(2):
                        b = 2 * hv + bb
                        nc.vector.scalar_tensor_tensor(
                            dense(xo_rhs, kc)[:, b], ps4[:, bb],
                            b_sp_c[:, kc:kc + 1],
                            int2(g_p, kc)[:, b],
                            op0=OP.add, op1=OP.mult)
        if SMAX >= 9:
            for mc in range(NCC):
                for hv in range(HV):
                    ps = pp_mm.tile([128, 512], fp32, tag="mm", name=f"wo{mc}{hv}")
                    for kc in range(NCC):
                        nc.tensor.matmul(
                            ps[:], w_out_t[:, kc, mc * 128:(mc + 1) * 128],
                            xo_rhs[:, kc, hv * 512:(hv + 1) * 512],
                            start=(kc == 0), stop=(kc == NCC - 1))
                    nc.vector.scalar_tensor_tensor(
                        out1[:, mc, hv * 512:(hv + 1) * 512],
                        ps[:], b_out_c[:, mc:mc + 1],
                        x_cm[:, mc, hv * 512:(hv + 1) * 512],
                        op0=OP.add, op1=OP.add)

    # ---------------- J: LN2 ----------------
    # Per-token stats land directly in [128, 8] token-partition layout via
    # data-stationary matmuls against ones/C, so the rstd chain runs on 128
    # partitions (the old [1,1024] chain cost ~15us serial).
    if SMAX >= 10:
        o1b = xpos  # reuse again: bf16 copy of out1
        for kc in range(NCC):
            nc.scalar.copy(o1b[:, kc, :], out1[:, kc, :])
            nc.vector.tensor_tensor(u_f[:, kc, :], o1b[:, kc, :], o1b[:, kc, :],
                                    op=OP.mult)   # squares into u_f
        psT = pp_sm.tile([128, 16], fp32, tag="sm", name="psT")
        for i in range(NTOK // 128):
            for kc in range(NCC):
                nc.tensor.matmul(psT[:, i:i + 1],
                                 o1b[:, kc, i * 128:(i + 1) * 128], ones_c[:],
                                 start=(kc == 0), stop=(kc == NCC - 1))
            for kc in range(NCC):
                nc.tensor.matmul(psT[:, 8 + i:9 + i],
                                 u_f[:, kc, i * 128:(i + 1) * 128], ones_c[:],
                                 start=(kc == 0), stop=(kc == NCC - 1))
        nc.vector.tensor_copy(ln2m[:], psT[:, 0:8])
        nc.vector.tensor_tensor(ln2v[:], ln2m[:], ln2m[:], op=OP.mult)
        nc.vector.tensor_tensor(ln2v[:], psT[:, 8:16], ln2v[:], op=OP.subtract)
        nc.vector.tensor_scalar(ln2v[:], ln2v[:], EPS, None, op0=OP.add)
        nc.scalar.sqrt(ln2v[:], ln2v[:])
        nc.vector.reciprocal(ln2v[:], ln2v[:])          # rstd [128, 8]
        nc.vector.tensor_copy(ln2b[:, 0:8], ln2v[:])
        nc.vector.scalar_tensor_tensor(
            ln2b[:, 8:16], ln2m[:], -1.0, ln2v[:], op0=OP.mult, op1=OP.mult)
        psb = pp_tr.tile([16, 128], fp32, tag="tr", name="psb")
        nc.tensor.matmul(psb[:], ln2b[:], ident[:], start=True, stop=True)
        nc.scalar.copy(rsb[:], psb[:])
        # stg is dead after stage A; borrow one partition row as the
        # [1, 2048] staging row for the per-token LN2 scale/shift
        rsrow = stg[:].rearrange("p a b -> p (a b)")[0:1, 0:2048]
        nc.sync.dma_start(
            rsrow.rearrange("o (si p) -> o si p", si=16), rsb[:])
        rhsS = rsrow[:, 0:1024]
        rhsM = rsrow[:, 1024:2048]
        yn = z_f  # reuse z tile as yn
        for kc in range(NCC):
            for hv in range(HV):
                psS = pp_mm.tile([128, 512], fp32, tag="mm", name=f"lnS{kc}{hv}")
                nc.tensor.matmul(psS[:], g2_t[0:1, kc, :],
                                 rhsS[:, hv * 512:(hv + 1) * 512],
                                 start=True, stop=True)
                psB = pp_mm.tile([128, 512], fp32, tag="mm", name=f"lnB{kc}{hv}")
                nc.tensor.matmul(psB[:], g2_t[0:1, kc, :],
                                 rhsM[:, hv * 512:(hv + 1) * 512],
                                 start=True, stop=True)
                nc.vector.tensor_tensor(
                    yn[:, kc, hv * 512:(hv + 1) * 512],
                    o1b[:, kc, hv * 512:(hv + 1) * 512], psS[:], op=OP.mult)
                nc.vector.scalar_tensor_tensor(
                    yn[:, kc, hv * 512:(hv + 1) * 512],
                    yn[:, kc, hv * 512:(hv + 1) * 512], be2_c[:, kc:kc + 1],
                    psB[:], op0=OP.add, op1=OP.add)

    # ---------------- K: MLP ----------------
    if SMAX >= 11:
        for jc in range(NHC):
            for hv in range(HV):
                ps = pp_mm.tile([128, 512], fp32, tag="mm", name=f"w1_{jc}{hv}")
                for kc in range(NCC):
                    nc.tensor.matmul(
                        ps[:], w1_t[:, kc, jc * 128:(jc + 1) * 128],
                        yn[:, kc, hv * 512:(hv + 1) * 512],
                        start=(kc == 0), stop=(kc == NCC - 1))
                ps4 = ps[:].rearrange("p (b h w) -> p b h w", b=2, h=H, w=W)
                for bb in range(2):
                    # DVE evac (fp8 write) keeps the Scalar engine free for
                    # the dwconv gelu evacs
                    nc.vector.tensor_scalar(
                        pad1(h1p, jc)[:, 2 * hv + bb, 1:1 + H, 1:1 + W],
                        ps4[:, bb], b1_c[:, jc:jc + 1], None, op0=OP.add)
        # depthwise 3x3 on HID channels: fp8e4 DoubleRow, two diagonal taps
        # contracted per pass (host scales the kernel by DW8S; the gelu evac
        # compensates via its activation scale).
        PAIRS = [((0, 0), (0, 1)), ((0, 2), (1, 0)),
                 ((1, 1), (1, 2)), ((2, 0), (2, 1))]
        SINGLE = (2, 2)
        DR = mybir.MatmulPerfMode.DoubleRow
        for jc in range(NHC):
            for b in range(BL):
                psw = pp_mm.tile([128, 512], fp32, tag="mm", name=f"cdw{jc}{b}")
                ps = psw[:, 0:256]
                rhs1 = pad1(h1p, jc)[:, b, SINGLE[0]:SINGLE[0] + H,
                                     SINGLE[1]:SINGLE[1] + W]
                nc.tensor.matmul(ps[:], ddw1_t[:, jc, :], rhs1,
                                 start=True, stop=True, skip_group_check=True)
                for pi, ((i0, j0), (i1, j1)) in enumerate(PAIRS):
                    base = pad1(h1p, jc)[:, b, i0:i0 + H, j0:j0 + W]
                    delta = (i1 - i0) * W1P + (j1 - j0)
                    rhs = bass.AP(base.tensor, base.offset,
                                  [list(base.ap[0]), [delta, 2],
                                   list(base.ap[1]), list(base.ap[2])])
                    nc.tensor.matmul(ps[:], ddw8_t[:, pi, jc, :, :], rhs,
                                     start=False, stop=(pi == 3), perf_mode=DR,
                                     skip_group_check=True)
                nc.scalar.activation(
                    pad1(h1p, jc)[:, b, 1:1 + H, 1:1 + W],
                    ps[:].rearrange("p (h w) -> p h w", h=H),
                    AF.Gelu_apprx_tanh, bias=bdw_c[:, jc:jc + 1],
                    scale=1.0 / DW8S)
        # W2 + output transpose/store, interleaved per token-half so the
        # first half's store overlaps the second half's W2 matmuls
        oh = sg_f   # dead by stage L, reuse
        ol = u_f
        out_s = pool.tile([128, NTOK // 128, C], fp32, name="out_s")
        out_dv = out_d[:].rearrange("(i p) c -> p i c", p=128)
        for hv in range(HV):
            sl = slice(hv * 512, (hv + 1) * 512)
            for mc in range(NCC):
                ps = pp_mm.tile([128, 512], fp32, tag="mm", name=f"w2_{mc}{hv}")
                for jc in range(NHC):
                    nc.tensor.matmul(
                        ps[:], w2_t[:, jc, mc * 128:(mc + 1) * 128],
                        int1(h1p, jc)[:, 2 * hv:2 * hv + 2],
                        start=(jc == 0), stop=(jc == NHC - 1))
                nc.vector.scalar_tensor_tensor(
                    x_cm[:, mc, sl], ps[:], b2_c[:, mc:mc + 1],
                    out1[:, mc, sl], op0=OP.add, op1=OP.add)
                nc.scalar.copy(oh[:, mc, sl], x_cm[:, mc, sl])
                nc.vector.scalar_tensor_tensor(
                    ol[:, mc, sl], oh[:, mc, sl], -1.0, x_cm[:, mc, sl],
                    op0=OP.mult, op1=OP.add)
            for i in range(hv * 4, hv * 4 + 4):
                for mc in range(NCC):
                    pt = pp_tr.tile([128, 128], fp32, tag="tr",
                                    name=f"tro{i}_{mc}")
                    nc.tensor.matmul(pt[:], oh[:, mc, i * 128:(i + 1) * 128],
                                     ident[:], start=True, stop=False)
                    nc.tensor.matmul(pt[:], ol[:, mc, i * 128:(i + 1) * 128],
                                     ident[:], start=False, stop=True)
                    nc.scalar.copy(out_s[:, i, mc * 128:(mc + 1) * 128], pt[:])
                nc.sync.dma_start(out_dv[:, i:i + 1], out_s[:, i:i + 1])

    ctx.close()


# ------------------------------------------------------------------
# host side
# ------------------------------------------------------------------

def _diagify(k2d, nchunks):
    """k2d: (KH, KW, 1, Cn) -> (KH*KW, nchunks, 128, 128) bf16 diagonals."""
    kh, kw = k2d.shape[0], k2d.shape[1]
    out = np.zeros((kh * kw, nchunks, 128, 128), dtype=BF16)
    idx = np.arange(128)
    for t in range(kh * kw):
        vals = k2d[t // kw, t % kw, 0].astype(np.float32)
        for c in range(nchunks):
            out[t, c, idx, idx] = vals[c * 128:(c + 1) * 128].astype(BF16)
    return out


def _prep_shared(w):
    """Build the shared (weight) input map from the raw input dict."""
    f32 = np.float32
    m = {}
    def pm(a):  # [k,128,...] -> [128,k,...] contiguous
        return np.ascontiguousarray(np.moveaxis(a, 1, 0))

    m["w_in"] = pm(w["W_in"].astype(f32).reshape(NCC, 128, C)).astype(BF16)
    m["w_a"] = pm(w["W_a"].astype(f32).reshape(NCC, 128, C)).astype(BF16)
    m["w_g"] = pm(w["W_g"].astype(f32).reshape(NCC, 128, C)).astype(BF16)
    m["w_out"] = pm(w["W_out"].astype(f32).reshape(NCC, 128, C)).astype(BF16)
    m["w1"] = pm(w["W1"].astype(f32).reshape(NCC, 128, HID)).astype(BF16)
    m["w2"] = pm(w["W2"].astype(f32).reshape(NHC, 128, C)).astype(BF16)
    m["wg1"] = pm(w["Wg1"].astype(f32).reshape(2 * NCC, 128, GH)).astype(BF16)
    m["wg2"] = w["Wg2"].astype(f32).reshape(GH, 1).astype(BF16)
    m["dpos"] = np.ascontiguousarray(
        _diagify(np.asarray(w["w_pos"]), NCC).transpose(2, 0, 1, 3))
    m["dsp"] = np.ascontiguousarray(
        _diagify(np.asarray(w["k_sp"]), NCC).transpose(2, 0, 1, 3))
    # fp8 DoubleRow tap-pair diagonals for the MLP depthwise conv, scaled by
    # DW8S (compensated in the gelu evac)
    wdw = np.asarray(w["wdw"], f32) * DW8S     # (3,3,1,HID)
    idx = np.arange(128)
    pairs = [((0, 0), (0, 1)), ((0, 2), (1, 0)),
             ((1, 1), (1, 2)), ((2, 0), (2, 1))]
    ddw8 = np.zeros((128, 4, NHC, 2, 128), dtype=FP8)
    for pi, (ta, tb) in enumerate(pairs):
        for k, t in enumerate((ta, tb)):
            vals = wdw[t[0], t[1], 0].reshape(NHC, 128)
            for jc in range(NHC):
                ddw8[idx, pi, jc, k, idx] = vals[jc].astype(FP8)
    ddw1 = np.zeros((128, NHC, 128), dtype=FP8)
    vals = wdw[2, 2, 0].reshape(NHC, 128)
    for jc in range(NHC):
        ddw1[idx, jc, idx] = vals[jc].astype(FP8)
    m["ddw8"] = ddw8
    m["ddw1"] = ddw1
    for src, dst, n in [("b_in", "b_in", NCC), ("b_a", "b_a", NCC),
                        ("b_g", "b_g", NCC), ("b_sp", "b_sp", NCC),
                        ("b_out", "b_out", NCC), ("b2", "b2", NCC),
                        ("gamma1", "gamma1", NCC), ("beta1", "beta1", NCC),
                        ("b1", "b1", NHC), ("bdw", "bdw", NHC)]:
        m[dst] = np.ascontiguousarray(np.asarray(w[src], f32).reshape(n, 128).T)
    m["b_pos"] = np.ascontiguousarray(
        np.asarray(w["b_pos"], f32).reshape(NCC, 128).T)
    m["g2r"] = np.asarray(w["gamma2"], f32).reshape(1, NCC, 128).astype(BF16)
    m["be2"] = np.ascontiguousarray(
        np.asarray(w["beta2"], f32).reshape(NCC, 128).T)
    m["bg1"] = np.asarray(w["bg1"], f32).reshape(GH, 1)
    m["bg2"] = np.asarray(w["bg2"], f32).reshape(1, 1)
    prior = np.zeros((T,), f32)
    prior[-1] = 4.0
    m["prior"] = np.tile(prior, BL)[None, :]
    rp = RHO ** (np.arange(1, T + 1, dtype=f32))
    m["rhopow"] = np.tile(rp, BL)[None, :].astype(f32)
    m["rif"] = np.broadcast_to((-rp / float(HWN))[None, :], (128, T)).astype(f32)
    m["rif"] = np.ascontiguousarray(m["rif"])
    return m


TRACE = False       # set True (e.g. from test.py) to capture an NTFF profile
LAST_RES = None


def kernel(**inputs):
    global _PROG, LAST_RES
    from concourse.bass_utils import run_bass_kernel_spmd

    if _PROG is None:
        _PROG = _build_program()
    nc = _PROG

    shared = _prep_shared(inputs)
    x = np.asarray(inputs["x"], np.float32)
    in_maps = []
    for i in range(NCORES):
        im = dict(shared)
        xs = np.ascontiguousarray(x[i * BL:(i + 1) * BL].reshape(NTOK, C))
        xhi = xs.astype(BF16)
        im["x_hi"] = xhi
        im["x_lo"] = (xs - xhi.astype(np.float32)).astype(BF16)
        in_maps.append(im)

    res = run_bass_kernel_spmd(nc, in_maps, core_ids=list(range(NCORES)),
                               trace=TRACE)
    LAST_RES = res
    out = np.concatenate(
        [r["out"].reshape(BL, H, W, C) for r in res.results], axis=0)
    return out
